# revision 22
# baseline (speedup 1.0000x reference)
"""Trainium2 Bass kernel: causal self-attention (B=4, T=2048, D=1024, H=16).

Sharding: 8 cores = (batch b in 0..3) x (head-group hg in 0..1).
Each core handles one batch element and 8 heads (CL=512 channels).

Fused chunk pipeline (per 512-token chunk c):
  - load + PE-transpose x tiles -> xT chunk (bf16)
  - V projection for the chunk -> vA tiles [keys, 8*65] (ones col = denom)
  - per head-pair ct: q/k projections for the chunk, then immediately
    causal attention for query block m=c (all keys 0..c available):
      ST = kT x qT (bf16, PSUM bf16) -> ACT exp -> Pool tri-mask ->
      PV accumulate [65, 512] with ones row giving the denominator
    normalization: DVE reciprocal of denom row + Pool partition
    broadcast + DVE multiply -> AT (bf16)
  - output projection m=c -> yTp [D, T] (bf16), DMA out

This overlaps the exp stream (ACT, ~.14ms total) with projection matmuls
(PE) instead of serializing phases. All matmuls bf16 (1 cycle/col at any
N, so the narrow diag/PV matmuls avoid the fp32r 4x penalty).

Host combines: y[b] = (yTp[2b] + yTp[2b+1])^T + b_out.
"""

import numpy as np

B, T, D = 4, 2048, 1024
H, DH = 16, 64
HL, CL = 8, 512          # local heads / channels per core
NT = T // 128            # 16 token tiles
NKT = D // 128           # 8 contraction tiles for QKV
NCH = T // 512           # 4 token chunks == query blocks

_CACHE = {}
PHASE_MARKS = []


def _mark(nc, phase):
    PHASE_MARKS.append((phase, nc.next_id()))


def build_program(reps=1, timing=False):
    import concourse.bacc as bacc
    import concourse.tile as tile
    from concourse import mybir

    F32 = mybir.dt.float32
    BF16 = mybir.dt.bfloat16
    AF = mybir.ActivationFunctionType

    nc = bacc.Bacc("TRN2", target_bir_lowering=False, debug=False)

    xb = nc.dram_tensor("xb", [T, D], BF16, kind="ExternalInput")
    wq = nc.dram_tensor("wq", [D, CL], BF16, kind="ExternalInput")
    wk = nc.dram_tensor("wk", [D, CL], BF16, kind="ExternalInput")
    wv = nc.dram_tensor("wv", [D, CL], BF16, kind="ExternalInput")
    wo = nc.dram_tensor("wo", [CL, D], BF16, kind="ExternalInput")
    bq = nc.dram_tensor("bq", [CL], F32, kind="ExternalInput")
    bk = nc.dram_tensor("bk", [CL], F32, kind="ExternalInput")
    bva = nc.dram_tensor("bva", [HL * 65], F32, kind="ExternalInput")
    tri = nc.dram_tensor("tri", [128, 128], BF16, kind="ExternalInput")
    ident = nc.dram_tensor("ident", [128, 128], BF16, kind="ExternalInput")
    if timing:
        ytp = nc.dram_tensor("ytp", [D, T], BF16)
        done = nc.dram_tensor("done", [1, 4], F32, kind="ExternalOutput")
    else:
        ytp = nc.dram_tensor("ytp", [D, T], BF16, kind="ExternalOutput")
        done = None

    with tile.TileContext(nc) as tc:
        with tc.tile_pool(name="consts", bufs=1) as consts, \
             tc.tile_pool(name="wgt", bufs=1) as wgt, \
             tc.tile_pool(name="kt", bufs=1) as ktpool, \
             tc.tile_pool(name="va", bufs=1) as vapool, \
             tc.tile_pool(name="xn", bufs=3) as xnpool, \
             tc.tile_pool(name="xt", bufs=2) as xtpool, \
             tc.tile_pool(name="qt", bufs=2) as qtpool, \
             tc.tile_pool(name="pt", bufs=6) as ptpool, \
             tc.tile_pool(name="at", bufs=2) as atpool, \
             tc.tile_pool(name="rows", bufs=4) as rows, \
             tc.tile_pool(name="bcsp", bufs=4) as bcspool, \
             tc.tile_pool(name="oy", bufs=4) as oypool, \
             tc.tile_pool(name="psT", bufs=2, space="PSUM") as psT, \
             tc.tile_pool(name="psO", bufs=2, space="PSUM") as psO, \
             tc.tile_pool(name="psP", bufs=2, space="PSUM") as psP:

            # ---------------- constants / weights ----------------
            # Pin the ACT table set that holds Copy+Exp so per-call
            # set-switch thrash never happens.
            nc.scalar.add_instruction(mybir.InstLoadActFuncSet(
                act_func_set_id=6,
                name=nc.get_next_instruction_name(),
                ins=[], outs=[]))
            # ACT-queue DMA order = first-use order: ident (transposes),
            # tiny biases, wv (V), wq/wk (qk), tri (first diag mask), wo.
            ident_sb = consts.tile([128, 128], BF16)
            nc.scalar.dma_start(out=ident_sb, in_=ident[:])
            bq_sb = consts.tile([128, 4], F32)
            nc.scalar.dma_start(out=bq_sb, in_=bq[:].rearrange("(c p) -> p c", p=128))
            bk_sb = consts.tile([128, 4], F32)
            nc.scalar.dma_start(out=bk_sb, in_=bk[:].rearrange("(c p) -> p c", p=128))
            bva_row = consts.tile([1, HL * 65], F32)
            nc.scalar.dma_start(out=bva_row, in_=bva[:].unsqueeze(0))
            bvat = consts.tile([128, HL * 65], F32)
            nc.gpsimd.partition_broadcast(bvat, bva_row)

            wv_sb = wgt.tile([128, NKT, CL], BF16, tag="wv")
            nc.scalar.dma_start(
                out=wv_sb, in_=wv[:].rearrange("(kt p) c -> p kt c", p=128))
            wq_sb = wgt.tile([128, NKT, CL], BF16, tag="wq")
            nc.scalar.dma_start(
                out=wq_sb, in_=wq[:].rearrange("(kt p) c -> p kt c", p=128))
            wk_sb = wgt.tile([128, NKT, CL], BF16, tag="wk")
            nc.scalar.dma_start(
                out=wk_sb, in_=wk[:].rearrange("(kt p) c -> p kt c", p=128))
            tri_sb = consts.tile([128, 128], BF16)
            nc.scalar.dma_start(out=tri_sb, in_=tri[:])
            wo_sb = wgt.tile([128, 4, D], BF16, tag="wo")
            nc.scalar.dma_start(
                out=wo_sb, in_=wo[:].rearrange("(cc p) d -> p cc d", p=128))

            # persistent activations
            kT = [ktpool.tile([128, T], BF16, name=f"kT{c}", tag=f"kT{c}")
                  for c in range(4)]
            vA = [vapool.tile([128, HL * 65], BF16, name=f"vA{t}",
                              tag=f"vA{t}")
                  for t in range(NT)]

            for _rep in range(reps):
                def emit_outproj(mch, ATm):
                    _mark(nc, f"o:c{mch}")
                    for cot in range(8):
                        psy = psP.tile([128, 512], F32, tag="psP")
                        for cc in range(4):
                            nc.tensor.matmul(
                                psy,
                                wo_sb[:, cc, cot * 128:(cot + 1) * 128],
                                ATm[cc],
                                start=(cc == 0), stop=(cc == 3))
                        oy = oypool.tile([128, 512], BF16, tag="oy")
                        nc.vector.tensor_copy(oy, psy)
                        nc.sync.dma_start(
                            out=ytp[cot * 128:(cot + 1) * 128,
                                    mch * 512:(mch + 1) * 512],
                            in_=oy)

                AT_prev = None
                pend = []  # deferred AT normalizations: (otpA, otpB, ATc)

                def emit_norms():
                    while pend:
                        otpA, otpB, ATc = pend.pop(0)
                        for ro, otp in ((0, otpA), (64, otpB)):
                            rcr = rows.tile([1, 512], F32, tag="rcr")
                            with nc.allow_low_precision(
                                    reason="softmax denom to bf16"):
                                nc.vector.reciprocal(rcr, otp[64:65, :])
                            bcs = bcspool.tile([64, 512], F32, tag="bcs")
                            nc.gpsimd.partition_broadcast(bcs, rcr)
                            nc.vector.tensor_mul(
                                ATc[ro:ro + 64, :], otp[0:64, :], bcs)

                for ch in range(NCH):
                    _mark(nc, f"x:c{ch}")
                    # ---- transpose x + V projection for this chunk ----
                    xT = xtpool.tile([128, NKT, 512], BF16, tag="xT")
                    for lt in range(4):
                        tt = 4 * ch + lt
                        xn = xnpool.tile([128, D], BF16, tag="xn")
                        nc.sync.dma_start(
                            out=xn, in_=xb[tt * 128:(tt + 1) * 128, :])
                        for g in range(2):
                            pst = psP.tile([128, 512], BF16, tag="psP")
                            for j4 in range(4):
                                kt = 4 * g + j4
                                nc.tensor.transpose(
                                    pst[:, j4 * 128:(j4 + 1) * 128],
                                    xn[:, kt * 128:(kt + 1) * 128],
                                    ident_sb)
                            # evac on ACT: keeps the copy off the DVE queue,
                            # which would otherwise delay next-chunk qk
                            # behind the AT normalization chain
                            nc.scalar.activation(
                                out=xT[:, 4 * g:4 * g + 4,
                                       lt * 128:(lt + 1) * 128],
                                in_=pst.rearrange("p (a b) -> p a b", a=4),
                                func=AF.Copy)
                        # ones only in the per-head denominator columns; the
                        # V evac fills the other 8*64
                        nc.vector.memset(
                            vA[tt].rearrange("p (a b) -> p a b", b=65)[:, :, 64:65],
                            1.0)
                        vps = psP.tile([128, 512], F32, tag="psP")
                        for kt in range(NKT):
                            nc.tensor.matmul(
                                vps,
                                xT[:, kt, lt * 128:(lt + 1) * 128],
                                wv_sb[:, kt, :],
                                start=(kt == 0), stop=(kt == NKT - 1))
                        nc.vector.tensor_tensor(
                            out=vA[tt].rearrange(
                                "p (a b) -> p a b", b=65)[:, :, 0:64],
                            in0=vps.rearrange("p (a b) -> p a b", a=8),
                            in1=bvat.rearrange(
                                "p (a b) -> p a b", b=65)[:, :, 0:64],
                            op=mybir.AluOpType.add)

                    # ---- flush the previous head-pair's deferred
                    # normalization, then the previous block's projection ----
                    emit_norms()
                    if AT_prev is not None:
                        emit_outproj(ch - 1, AT_prev)

                    # ---- per head-pair: q/k projection then attention ----
                    ATm = []
                    for ct in range(4):
                        _mark(nc, f"a:c{ch}h{ct}")
                        qTc = qtpool.tile([128, 512], BF16, tag=f"qT{ct}")
                        for isq in (True, False):
                            w_sb = wq_sb if isq else wk_sb
                            dst = (qTc if isq
                                   else kT[ct][:, ch * 512:(ch + 1) * 512])
                            bias = (bq_sb if isq else bk_sb)[:, ct:ct + 1]
                            ps = psP.tile([128, 512], F32, tag="psP")
                            for kt in range(NKT):
                                nc.tensor.matmul(
                                    ps, w_sb[:, kt, ct * 128:(ct + 1) * 128],
                                    xT[:, kt, :],
                                    start=(kt == 0), stop=(kt == NKT - 1))
                            nc.vector.tensor_scalar_add(
                                out=dst, in0=ps, scalar1=bias)

                        # previous pair's AT normalization goes to the DVE
                        # queue after this pair's qk evacs so ST isn't stalled
                        emit_norms()

                        # causal attention, query block m=ch, heads 2ct,2ct+1
                        otpA = psO.tile([65, 512], F32, tag="psO")
                        otpB = psO.tile([65, 512], F32, tag="psO")
                        njj = 4 * ch + 4
                        for jj in range(njj):
                            diag = jj < 4
                            if diag:
                                j, qoff = 4 * ch + jj, jj * 128
                            else:
                                j, qoff = jj - 4, 0
                            js = slice(j * 128, (j + 1) * 128)
                            stD = psT.tile([128, 1024], F32, tag="psT")
                            nc.tensor.matmul(
                                stD[:, qoff:512],
                                kT[ct][0:64, js],
                                qTc[0:64, qoff:512],
                                start=True, stop=True)
                            nc.tensor.matmul(
                                stD[:, 512 + qoff:1024],
                                kT[ct][64:128, js],
                                qTc[64:128, qoff:512],
                                start=True, stop=True)
                            ptD = ptpool.tile([128, 1024], BF16, tag="pt")
                            if diag:
                                pt3 = ptD.rearrange("p (h q) -> p h q", h=2)
                                st3 = stD.rearrange("p (h q) -> p h q", h=2)
                                nc.scalar.activation(
                                    out=pt3[:, :, qoff:512],
                                    in_=st3[:, :, qoff:512],
                                    func=AF.Exp, scale=0.125)
                                nc.vector.tensor_mul(
                                    pt3[:, :, qoff:qoff + 128],
                                    pt3[:, :, qoff:qoff + 128],
                                    tri_sb.unsqueeze(1).broadcast_to(
                                        [128, 2, 128]))
                            else:
                                nc.scalar.activation(
                                    out=ptD, in_=stD,
                                    func=AF.Exp, scale=0.125)
                            hA, hB = 2 * ct, 2 * ct + 1
                            nc.tensor.matmul(
                                otpA[:, qoff:512],
                                vA[j][:, hA * 65:(hA + 1) * 65],
                                ptD[:, qoff:512],
                                start=(jj == 0), stop=(jj == njj - 1))
                            nc.tensor.matmul(
                                otpB[:, qoff:512],
                                vA[j][:, hB * 65:(hB + 1) * 65],
                                ptD[:, 512 + qoff:1024],
                                start=(jj == 0), stop=(jj == njj - 1))
                        # normalization (by the ones-column denominator row)
                        # is deferred: see emit_norms
                        ATc = atpool.tile([128, 512], BF16, tag=f"AT{ct}")
                        ATm.append(ATc)
                        pend.append((otpA, otpB, ATc))

                    AT_prev = ATm
                emit_norms()
                emit_outproj(NCH - 1, AT_prev)
            if done is not None:
                dn = consts.tile([1, 4], F32)
                nc.vector.memset(dn, 1.0)
                nc.sync.dma_start(out=done[:], in_=dn)
    nc.compile()
    return nc


def _consts():
    import ml_dtypes
    bf16 = np.dtype(ml_dtypes.bfloat16)
    tri = (np.arange(128)[None, :] >= np.arange(128)[:, None]).astype(bf16)
    ident = np.eye(128, dtype=np.float32).astype(bf16)
    return tri, ident


def make_in_maps(x, W_qkv, b_qkv, W_out):
    import ml_dtypes
    bf16 = np.dtype(ml_dtypes.bfloat16)
    x = np.asarray(x, dtype=np.float32)
    W_qkv = np.asarray(W_qkv, dtype=np.float32)
    b_qkv = np.asarray(b_qkv, dtype=np.float32)
    W_out = np.asarray(W_out, dtype=np.float32)
    tri, ident = _consts()
    in_maps = []
    for core in range(8):
        b, hg = core // 2, core % 2
        cs = hg * CL
        bv = b_qkv[2 * D + cs:2 * D + cs + CL]
        bva = np.zeros(HL * 65, dtype=np.float32)
        bva.reshape(HL, 65)[:, 0:64] = bv.reshape(HL, 64)
        in_maps.append({
            "xb": np.ascontiguousarray(x[b]).astype(bf16),
            "wq": np.ascontiguousarray(W_qkv[:, cs:cs + CL]).astype(bf16),
            "wk": np.ascontiguousarray(
                W_qkv[:, D + cs:D + cs + CL]).astype(bf16),
            "wv": np.ascontiguousarray(
                W_qkv[:, 2 * D + cs:2 * D + cs + CL]).astype(bf16),
            "wo": np.ascontiguousarray(W_out[cs:cs + CL, :]).astype(bf16),
            "bq": np.ascontiguousarray(b_qkv[cs:cs + CL]),
            "bk": np.ascontiguousarray(b_qkv[D + cs:D + cs + CL]),
            "bva": bva,
            "tri": tri,
            "ident": ident,
        })
    return in_maps


def combine_outputs(results, b_out):
    b_out = np.asarray(b_out, dtype=np.float32)
    y = np.empty((B, T, D), dtype=np.float32)
    for b in range(B):
        yt = (results[2 * b]["ytp"].astype(np.float32)
              + results[2 * b + 1]["ytp"].astype(np.float32))
        y[b] = yt.T + b_out
    return y


def kernel(x, W_qkv, b_qkv, W_out, b_out):
    from concourse.bass_utils import run_bass_kernel_spmd
    if "nc" not in _CACHE:
        _CACHE["nc"] = build_program()
    nc = _CACHE["nc"]
    in_maps = make_in_maps(x, W_qkv, b_qkv, W_out)
    res = run_bass_kernel_spmd(nc, in_maps, list(range(8)))
    return combine_outputs(res.results, b_out)


# revision 23
# speedup vs baseline: 1.2537x; 1.2537x over previous
"""Trainium2 Bass kernel: causal self-attention (B=4, T=2048, D=1024, H=16).

Sharding: 8 cores = (batch b in 0..3) x (head-group hg in 0..1).
Each core handles one batch element and 8 heads (CL=512 channels).

Fused chunk pipeline (per 512-token chunk c):
  - load + PE-transpose x tiles -> xT chunk (bf16)
  - V projection for the chunk -> vA tiles [keys, 8*65] (ones col = denom)
  - per head-pair ct: q/k projections for the chunk, then immediately
    causal attention for query block m=c (all keys 0..c available):
      ST = kT x qT (bf16, PSUM bf16) -> ACT exp -> Pool tri-mask ->
      PV accumulate [65, 512] with ones row giving the denominator
    normalization: DVE reciprocal of denom row + Pool partition
    broadcast + DVE multiply -> AT (bf16)
  - output projection m=c -> yTp [D, T] (bf16), DMA out

This overlaps the exp stream (ACT, ~.14ms total) with projection matmuls
(PE) instead of serializing phases. All matmuls bf16 (1 cycle/col at any
N, so the narrow diag/PV matmuls avoid the fp32r 4x penalty).

Host combines: y[b] = (yTp[2b] + yTp[2b+1])^T + b_out.
"""

import numpy as np

B, T, D = 4, 2048, 1024
H, DH = 16, 64
HL, CL = 8, 512          # local heads / channels per core
NT = T // 128            # 16 token tiles
NKT = D // 128           # 8 contraction tiles for QKV
NCH = T // 512           # 4 token chunks == query blocks

_CACHE = {}
PHASE_MARKS = []


def _mark(nc, phase):
    PHASE_MARKS.append((phase, nc.next_id()))


def build_program(reps=1, timing=False):
    import concourse.bacc as bacc
    import concourse.tile as tile
    from concourse import mybir

    F32 = mybir.dt.float32
    BF16 = mybir.dt.bfloat16
    AF = mybir.ActivationFunctionType

    nc = bacc.Bacc("TRN2", target_bir_lowering=False, debug=False)

    xb = nc.dram_tensor("xb", [T, D], BF16, kind="ExternalInput")
    wq = nc.dram_tensor("wq", [D, CL], BF16, kind="ExternalInput")
    wk = nc.dram_tensor("wk", [D, CL], BF16, kind="ExternalInput")
    wv = nc.dram_tensor("wv", [D, CL], BF16, kind="ExternalInput")
    wo = nc.dram_tensor("wo", [CL, D], BF16, kind="ExternalInput")
    bq = nc.dram_tensor("bq", [CL], F32, kind="ExternalInput")
    bk = nc.dram_tensor("bk", [CL], F32, kind="ExternalInput")
    bva = nc.dram_tensor("bva", [HL * 65], F32, kind="ExternalInput")
    tri = nc.dram_tensor("tri", [128, 128], BF16, kind="ExternalInput")
    ident = nc.dram_tensor("ident", [128, 128], BF16, kind="ExternalInput")
    if timing:
        ytp = nc.dram_tensor("ytp", [D, T], BF16)
        done = nc.dram_tensor("done", [1, 4], F32, kind="ExternalOutput")
    else:
        ytp = nc.dram_tensor("ytp", [D, T], BF16, kind="ExternalOutput")
        done = None

    with tile.TileContext(nc) as tc:
        with tc.tile_pool(name="consts", bufs=1) as consts, \
             tc.tile_pool(name="wgt", bufs=1) as wgt, \
             tc.tile_pool(name="kt", bufs=1) as ktpool, \
             tc.tile_pool(name="va", bufs=1) as vapool, \
             tc.tile_pool(name="xn", bufs=3) as xnpool, \
             tc.tile_pool(name="xt", bufs=2) as xtpool, \
             tc.tile_pool(name="qt", bufs=3) as qtpool, \
             tc.tile_pool(name="pt", bufs=8) as ptpool, \
             tc.tile_pool(name="at", bufs=3) as atpool, \
             tc.tile_pool(name="rows", bufs=4) as rows, \
             tc.tile_pool(name="bcsp", bufs=4) as bcspool, \
             tc.tile_pool(name="oy", bufs=6) as oypool, \
             tc.tile_pool(name="psT", bufs=2, space="PSUM") as psT, \
             tc.tile_pool(name="psO", bufs=2, space="PSUM") as psO, \
             tc.tile_pool(name="psP", bufs=2, space="PSUM") as psP:

            # ---------------- constants / weights ----------------
            # Pin the ACT table set that holds Copy+Exp so per-call
            # set-switch thrash never happens.
            nc.scalar.add_instruction(mybir.InstLoadActFuncSet(
                act_func_set_id=6,
                name=nc.get_next_instruction_name(),
                ins=[], outs=[]))
            # ACT-queue DMA order = first-use order: ident (transposes),
            # tiny biases, wv (V), wq/wk (qk), tri (first diag mask), wo.
            ident_sb = consts.tile([128, 128], BF16)
            nc.scalar.dma_start(out=ident_sb, in_=ident[:])
            bq_sb = consts.tile([128, 4], F32)
            nc.scalar.dma_start(out=bq_sb, in_=bq[:].rearrange("(c p) -> p c", p=128))
            bk_sb = consts.tile([128, 4], F32)
            nc.scalar.dma_start(out=bk_sb, in_=bk[:].rearrange("(c p) -> p c", p=128))
            bva_row = consts.tile([1, HL * 65], F32)
            nc.scalar.dma_start(out=bva_row, in_=bva[:].unsqueeze(0))
            bvat = consts.tile([128, HL * 65], F32)
            nc.gpsimd.partition_broadcast(bvat, bva_row)

            wv_sb = wgt.tile([128, NKT, CL], BF16, tag="wv")
            nc.scalar.dma_start(
                out=wv_sb, in_=wv[:].rearrange("(kt p) c -> p kt c", p=128))
            wq_sb = wgt.tile([128, NKT, CL], BF16, tag="wq")
            nc.scalar.dma_start(
                out=wq_sb, in_=wq[:].rearrange("(kt p) c -> p kt c", p=128))
            wk_sb = wgt.tile([128, NKT, CL], BF16, tag="wk")
            nc.scalar.dma_start(
                out=wk_sb, in_=wk[:].rearrange("(kt p) c -> p kt c", p=128))
            tri_sb = consts.tile([128, 128], BF16)
            nc.scalar.dma_start(out=tri_sb, in_=tri[:])
            wo_sb = wgt.tile([128, 4, D], BF16, tag="wo")
            nc.scalar.dma_start(
                out=wo_sb, in_=wo[:].rearrange("(cc p) d -> p cc d", p=128))

            # persistent activations
            kT = [ktpool.tile([128, T], BF16, name=f"kT{c}", tag=f"kT{c}")
                  for c in range(4)]
            vA = [vapool.tile([128, HL * 65], BF16, name=f"vA{t}",
                              tag=f"vA{t}")
                  for t in range(NT)]

            for _rep in range(reps):
                def emit_outproj(mch, ATm):
                    _mark(nc, f"o:c{mch}")
                    for cot in range(8):
                        psy = psP.tile([128, 512], F32, tag="psP")
                        for cc in range(4):
                            nc.tensor.matmul(
                                psy,
                                wo_sb[:, cc, cot * 128:(cot + 1) * 128],
                                ATm[cc],
                                start=(cc == 0), stop=(cc == 3))
                        oy = oypool.tile([128, 512], BF16, tag="oy")
                        nc.vector.tensor_copy(oy, psy)
                        nc.sync.dma_start(
                            out=ytp[cot * 128:(cot + 1) * 128,
                                    mch * 512:(mch + 1) * 512],
                            in_=oy)

                AT_prev = None
                pend = []  # deferred AT normalizations: (otpA, otpB, ATc)

                def emit_norms():
                    while pend:
                        otpA, otpB, ATc = pend.pop(0)
                        for ro, otp in ((0, otpA), (64, otpB)):
                            rcr = rows.tile([1, 512], F32, tag="rcr")
                            with nc.allow_low_precision(
                                    reason="softmax denom to bf16"):
                                nc.vector.reciprocal(rcr, otp[64:65, :])
                            bcs = bcspool.tile([64, 512], F32, tag="bcs")
                            nc.gpsimd.partition_broadcast(bcs, rcr)
                            nc.vector.tensor_mul(
                                ATc[ro:ro + 64, :], otp[0:64, :], bcs)

                for ch in range(NCH):
                    _mark(nc, f"x:c{ch}")
                    # ---- transpose x + V projection for this chunk ----
                    xT = xtpool.tile([128, NKT, 512], BF16, tag="xT")
                    for lt in range(4):
                        tt = 4 * ch + lt
                        xn = xnpool.tile([128, D], BF16, tag="xn")
                        nc.sync.dma_start(
                            out=xn, in_=xb[tt * 128:(tt + 1) * 128, :])
                        for g in range(2):
                            pst = psP.tile([128, 512], BF16, tag="psP")
                            for j4 in range(4):
                                kt = 4 * g + j4
                                nc.tensor.transpose(
                                    pst[:, j4 * 128:(j4 + 1) * 128],
                                    xn[:, kt * 128:(kt + 1) * 128],
                                    ident_sb)
                            # evac on ACT: keeps the copy off the DVE queue,
                            # which would otherwise delay next-chunk qk
                            # behind the AT normalization chain
                            nc.scalar.activation(
                                out=xT[:, 4 * g:4 * g + 4,
                                       lt * 128:(lt + 1) * 128],
                                in_=pst.rearrange("p (a b) -> p a b", a=4),
                                func=AF.Copy)
                        # ones only in the per-head denominator columns; the
                        # V evac fills the other 8*64
                        nc.vector.memset(
                            vA[tt].rearrange("p (a b) -> p a b", b=65)[:, :, 64:65],
                            1.0)
                        vps = psP.tile([128, 512], F32, tag="psP")
                        for kt in range(NKT):
                            nc.tensor.matmul(
                                vps,
                                xT[:, kt, lt * 128:(lt + 1) * 128],
                                wv_sb[:, kt, :],
                                start=(kt == 0), stop=(kt == NKT - 1))
                        nc.vector.tensor_tensor(
                            out=vA[tt].rearrange(
                                "p (a b) -> p a b", b=65)[:, :, 0:64],
                            in0=vps.rearrange("p (a b) -> p a b", a=8),
                            in1=bvat.rearrange(
                                "p (a b) -> p a b", b=65)[:, :, 0:64],
                            op=mybir.AluOpType.add)

                    # ---- flush the previous head-pair's deferred
                    # normalization, then the previous block's projection ----
                    emit_norms()
                    if AT_prev is not None:
                        emit_outproj(ch - 1, AT_prev)

                    # ---- per head-pair: q/k projection then attention ----
                    ATm = []
                    for ct in range(4):
                        _mark(nc, f"a:c{ch}h{ct}")
                        qTc = qtpool.tile([128, 512], BF16, tag=f"qT{ct}")
                        for isq in (True, False):
                            w_sb = wq_sb if isq else wk_sb
                            dst = (qTc if isq
                                   else kT[ct][:, ch * 512:(ch + 1) * 512])
                            bias = (bq_sb if isq else bk_sb)[:, ct:ct + 1]
                            ps = psP.tile([128, 512], F32, tag="psP")
                            for kt in range(NKT):
                                nc.tensor.matmul(
                                    ps, w_sb[:, kt, ct * 128:(ct + 1) * 128],
                                    xT[:, kt, :],
                                    start=(kt == 0), stop=(kt == NKT - 1))
                            nc.vector.tensor_scalar_add(
                                out=dst, in0=ps, scalar1=bias)

                        # previous pair's AT normalization goes to the DVE
                        # queue after this pair's qk evacs so ST isn't stalled
                        emit_norms()

                        # causal attention, query block m=ch, heads 2ct,2ct+1
                        otpA = psO.tile([65, 512], F32, tag="psO")
                        otpB = psO.tile([65, 512], F32, tag="psO")
                        njj = 4 * ch + 4
                        for jj in range(njj):
                            diag = jj < 4
                            if diag:
                                j, qoff = 4 * ch + jj, jj * 128
                            else:
                                j, qoff = jj - 4, 0
                            js = slice(j * 128, (j + 1) * 128)
                            stD = psT.tile([128, 1024], F32, tag="psT")
                            nc.tensor.matmul(
                                stD[:, qoff:512],
                                kT[ct][0:64, js],
                                qTc[0:64, qoff:512],
                                start=True, stop=True)
                            nc.tensor.matmul(
                                stD[:, 512 + qoff:1024],
                                kT[ct][64:128, js],
                                qTc[64:128, qoff:512],
                                start=True, stop=True)
                            ptD = ptpool.tile([128, 1024], BF16, tag="pt")
                            if diag:
                                pt3 = ptD.rearrange("p (h q) -> p h q", h=2)
                                st3 = stD.rearrange("p (h q) -> p h q", h=2)
                                nc.scalar.activation(
                                    out=pt3[:, :, qoff:512],
                                    in_=st3[:, :, qoff:512],
                                    func=AF.Exp, scale=0.125)
                                nc.vector.tensor_mul(
                                    pt3[:, :, qoff:qoff + 128],
                                    pt3[:, :, qoff:qoff + 128],
                                    tri_sb.unsqueeze(1).broadcast_to(
                                        [128, 2, 128]))
                            else:
                                nc.scalar.activation(
                                    out=ptD, in_=stD,
                                    func=AF.Exp, scale=0.125)
                            hA, hB = 2 * ct, 2 * ct + 1
                            nc.tensor.matmul(
                                otpA[:, qoff:512],
                                vA[j][:, hA * 65:(hA + 1) * 65],
                                ptD[:, qoff:512],
                                start=(jj == 0), stop=(jj == njj - 1))
                            nc.tensor.matmul(
                                otpB[:, qoff:512],
                                vA[j][:, hB * 65:(hB + 1) * 65],
                                ptD[:, 512 + qoff:1024],
                                start=(jj == 0), stop=(jj == njj - 1))
                        # normalization (by the ones-column denominator row)
                        # is deferred: see emit_norms
                        ATc = atpool.tile([128, 512], BF16, tag=f"AT{ct}")
                        ATm.append(ATc)
                        pend.append((otpA, otpB, ATc))

                    AT_prev = ATm
                emit_norms()
                emit_outproj(NCH - 1, AT_prev)
            if done is not None:
                dn = consts.tile([1, 4], F32)
                nc.vector.memset(dn, 1.0)
                nc.sync.dma_start(out=done[:], in_=dn)
    nc.compile()
    return nc


def _consts():
    import ml_dtypes
    bf16 = np.dtype(ml_dtypes.bfloat16)
    tri = (np.arange(128)[None, :] >= np.arange(128)[:, None]).astype(bf16)
    ident = np.eye(128, dtype=np.float32).astype(bf16)
    return tri, ident


def make_in_maps(x, W_qkv, b_qkv, W_out):
    import ml_dtypes
    bf16 = np.dtype(ml_dtypes.bfloat16)
    x = np.asarray(x, dtype=np.float32)
    W_qkv = np.asarray(W_qkv, dtype=np.float32)
    b_qkv = np.asarray(b_qkv, dtype=np.float32)
    W_out = np.asarray(W_out, dtype=np.float32)
    tri, ident = _consts()
    in_maps = []
    for core in range(8):
        b, hg = core // 2, core % 2
        cs = hg * CL
        bv = b_qkv[2 * D + cs:2 * D + cs + CL]
        bva = np.zeros(HL * 65, dtype=np.float32)
        bva.reshape(HL, 65)[:, 0:64] = bv.reshape(HL, 64)
        in_maps.append({
            "xb": np.ascontiguousarray(x[b]).astype(bf16),
            "wq": np.ascontiguousarray(W_qkv[:, cs:cs + CL]).astype(bf16),
            "wk": np.ascontiguousarray(
                W_qkv[:, D + cs:D + cs + CL]).astype(bf16),
            "wv": np.ascontiguousarray(
                W_qkv[:, 2 * D + cs:2 * D + cs + CL]).astype(bf16),
            "wo": np.ascontiguousarray(W_out[cs:cs + CL, :]).astype(bf16),
            "bq": np.ascontiguousarray(b_qkv[cs:cs + CL]),
            "bk": np.ascontiguousarray(b_qkv[D + cs:D + cs + CL]),
            "bva": bva,
            "tri": tri,
            "ident": ident,
        })
    return in_maps


def combine_outputs(results, b_out):
    b_out = np.asarray(b_out, dtype=np.float32)
    y = np.empty((B, T, D), dtype=np.float32)
    for b in range(B):
        yt = (results[2 * b]["ytp"].astype(np.float32)
              + results[2 * b + 1]["ytp"].astype(np.float32))
        y[b] = yt.T + b_out
    return y


def kernel(x, W_qkv, b_qkv, W_out, b_out):
    from concourse.bass_utils import run_bass_kernel_spmd
    if "nc" not in _CACHE:
        _CACHE["nc"] = build_program()
    nc = _CACHE["nc"]
    in_maps = make_in_maps(x, W_qkv, b_qkv, W_out)
    res = run_bass_kernel_spmd(nc, in_maps, list(range(8)))
    return combine_outputs(res.results, b_out)
